# revision 10
# baseline (speedup 1.0000x reference)
"""Trainium2 (8 NeuronCores) kernel for single-head causal attention.

Problem: x [8, 2048, 1024] f32; Wq/Wk/Wv [1024, 128] f32.
    q = x @ Wq ; k = x @ Wk ; v = x @ Wv          (per batch row)
    out = softmax(causal(q @ k^T / sqrt(128))) @ v  -> [8, 2048, 128] f32
Sharding: pure data-parallel - one batch row per NeuronCore, weights
replicated. No collectives.

Per-core algorithm (bf16 matmul inputs, f32 PSUM accumulation):
  Host supplies xT = x[b].T  [D, T] in bf16 (layout prep only).
  A) qT [H=128 part, T] / kT (split lo|hi tiles) with W-chunks stationary
     over 8 D-chunks, d-outer so PE tracks the xT DMA chunk by chunk.
     wq rides the SYNC ring ahead of the 8 xT chunk FIFO DMAs; wk/wv/mask
     ride the ACT ring - so neither weight gates the first chunk's q AND k
     matmuls (phase A is input-bandwidth-bound; order is everything).
  B) Scores TRANSPOSED: sT[k,q] = kT_j-block stationary @ qT, exact-causal,
     exp(scale*s) on ScalarE from PSUM into a CAUSAL-PACKED bf16 wT tile
     (17 maximal 1024-wide pieces, the last 1024 split 640/256/128 at
     segment boundaries so the final row-blocks' epilogues start early).
     No max-subtraction: |scale*s| <= ~7, safe in f32/bf16. Diagonal
     blocks get a multiplicative 0/1 strictly-causal mask into dw tiles
     on DVE (off the PE critical path).
  C) out[q,h] accumulates k-blocks jj<=i with wT stationary / v_aug moving
     (N=129; col 128 = ones => softmax denominator falls out free).
     C-group emission is DELAYED one piece: a group is emitted before the
     NEXT piece's exp, so its conservative whole-wT-tile dependency
     resolves to the PREVIOUS exp - already complete - and the PE stream
     never stalls on ScalarE. v-projections are spread one per piece as
     zero-dependency PE filler. C(15) is special-cased: its jj-th matmul
     is emitted as soon as segment jj is fully exp'd, so after the last
     (128-wide) exp only its diagonal matmul + epilogue remain.
     Normalize: DVE reciprocal + ScalarE copy-with-per-partition-scale
     (DVE tensor_scalar AP / stride-0 broadcasts are wrong on HW; ScalarE
     Reciprocal would thrash the activation table against Exp).
     Output DMAs are GROUPED {4,4,4,2,1,1} on the sync ring: fewer ring-
     slot-credit waits -> fewer event semaphores. The NEFF teardown clears
     every event semaphore at ~115ns each on every engine, so junction
     count is a first-class cost (~6.5us of the baseline's tail).

Engine discipline: hardware compute instructions carry at most ONE
semaphore wait; bacc legalizes extras into event-semaphore junctions
(move_matmul_waits_to_ldweights gives PE pairs two slots). Tile tracks
dependencies at TILE granularity. Post-build passes: (1) strip redundant
same-engine self-waits (PE/ACT/DVE complete in order); (2) strip the
out-scale mul's PE wait - it is transitively implied by its DVE wait
(the reciprocal it consumes already waited on the same PSUM stop-matmul).
"""

from contextlib import ExitStack

import ml_dtypes
import numpy as np

B, T, D, H = 8, 2048, 1024, 128
P = 128
DC = D // P  # 8 contraction chunks
TB = T // P  # 16 token blocks
QG = T // 512  # 4 512-wide token groups
SCALE = 1.0 / float(np.sqrt(H))

_CACHE = {}
LAST_RESULT = None


def _build():
    import concourse.bacc as bacc
    import concourse.mybir as mybir
    import concourse.tile as tile

    f32 = mybir.dt.float32
    bf16 = mybir.dt.bfloat16
    EXP = mybir.ActivationFunctionType.Exp
    MULT = mybir.AluOpType.mult

    nc = bacc.Bacc()
    xT_h = nc.declare_dram_parameter("xT", [D, T], bf16, isOutput=False)
    # weights host-prelayouted to [p, c, h]: contiguous 2048 B partition rows
    wq_h = nc.declare_dram_parameter("Wq", [P, DC, H], bf16, isOutput=False)
    wk_h = nc.declare_dram_parameter("Wk", [P, DC, H], bf16, isOutput=False)
    wv_h = nc.declare_dram_parameter("Wv", [P, DC, H], bf16, isOutput=False)
    mask_h = nc.declare_dram_parameter("mask", [P, P], bf16, isOutput=False)
    out_h = nc.declare_dram_parameter("out", [T, H], f32, isOutput=True)

    mul_names = []  # out-scale muls whose PE wait is transitively implied

    with tile.TileContext(nc) as tc:
        with ExitStack() as ctx:
            singles = ctx.enter_context(tc.tile_pool(name="singles", bufs=1))

            xT_sb = singles.tile([P, DC, T], bf16)
            wq_sb = singles.tile([P, DC, H], bf16)
            wk_sb = singles.tile([P, DC, H], bf16)
            wv_sb = singles.tile([P, DC, H], bf16)
            mask_sb = singles.tile([P, P], bf16)
            mask2_sb = singles.tile([P, P], bf16)
            qT_sb = singles.tile([P, T], bf16)
            kT_lo = singles.tile([P, T // 2], bf16)  # k-blocks 0..7
            kT_hi = singles.tile([P, T // 2], bf16)  # k-blocks 8..15
            v_sb = singles.tile([P, TB, 132], bf16)  # [...,128] = ones col
            wT_sb = singles.tile([P, 17408], bf16)  # causal-packed
            dw_sb = singles.tile([P, TB, P], bf16)  # masked diagonal blocks
            rec_all = singles.tile([P, TB], f32)
            ot_all = singles.tile([P, TB, H], f32)

            # SYNC ring: wq first (A1's q-side gate), then xT as 16 HALF-
            # chunk FIFO DMAs - same-queue DMAs stream sequentially so the
            # first half lands ~1.5us after wq and the d-outer projection
            # loop tracks the input at 256KB granularity; the last unit of
            # PE work gated on the final DMA byte is one 1024-col q/k pair
            # (~0.9us) instead of a whole chunk.
            nc.sync.dma_start(out=wq_sb, in_=wq_h[:])
            xT_ap = xT_h[:]
            for c in range(DC):
                for h in (0, 1):
                    nc.sync.dma_start(
                        out=xT_sb[:, c, h * 1024 : (h + 1) * 1024],
                        in_=xT_ap[c * P : (c + 1) * P, h * 1024 : (h + 1) * 1024],
                    )
            nc.scalar.dma_start(out=wk_sb, in_=wk_h[:])
            nc.scalar.dma_start(out=wv_sb, in_=wv_h[:])
            nc.scalar.dma_start(out=mask_sb, in_=mask_h[:])
            # ACT pre-touch: moves the mask's DMA wait onto a junction copy
            # so the per-j diag multiply's two deps merge into one ACT wait.
            nc.scalar.copy(mask2_sb, mask_sb)

            # PE warm-up fodder: HAM starts the PE throttled at 1.2 GHz and
            # needs ~3.4 us of sustained work to unthrottle; these dummies
            # run in the launch/DMA dead window. They write qps[0] BEFORE
            # its real accumulation group begins (start=True clears it).
            warm_sb = singles.tile([P, 512], bf16)
            nc.vector.memset(warm_sb, 0.0)

            # --- Phase A1: q/k projections, d-chunk OUTER so each xT
            # half-chunk is consumed as its DMA lands. qps/kps are SPLIT
            # lo|hi (2 banks each) so the first score piece's PSUM WAR
            # resolves against the EARLIEST eviction copy, not the last.
            with tc.tile_pool(name="psQK", bufs=1, space="PSUM") as psQK:
                qps_lo = psQK.tile([P, 1024], f32, tag="qps_lo")
                qps_hi = psQK.tile([P, 1024], f32, tag="qps_hi")
                kps_lo = psQK.tile([P, 1024], f32, tag="kps_lo")
                kps_hi = psQK.tile([P, 1024], f32, tag="kps_hi")
                for _ in range(8):
                    nc.tensor.matmul(
                        qps_lo[:, 0:512], warm_sb[:, 0:128], warm_sb,
                        start=True, stop=True,
                    )

                def qk_mm(is_q, g, c):
                    w_sb = wq_sb if is_q else wk_sb
                    acc = (
                        (qps_lo if g < 2 else qps_hi)
                        if is_q
                        else (kps_lo if g < 2 else kps_hi)
                    )
                    nc.tensor.matmul(
                        acc[:, (g % 2) * 512 : (g % 2 + 1) * 512],
                        w_sb[:, c, :],
                        xT_sb[:, c, g * 512 : (g + 1) * 512],
                        start=(c == 0),
                        stop=(c == DC - 1),
                    )

                for c in range(DC - 1):
                    for is_q in (True, False):
                        for g in (0, 1):  # half 0 of chunk c
                            qk_mm(is_q, g, c)
                    for is_q in (True, False):
                        for g in (2, 3):  # half 1 of chunk c
                            qk_mm(is_q, g, c)
                # Last chunk: evictions launched per half as it completes,
                # q-lo first (DVE) then k-lo (ACT) so the two copy engines
                # pipeline and the first score piece's gates (qT-lo cast,
                # kT-lo copy, qps-lo WAR) all resolve early.
                qk_mm(True, 0, DC - 1)
                qk_mm(True, 1, DC - 1)
                nc.vector.tensor_copy(qT_sb[:, 0:1024], qps_lo)
                qk_mm(False, 0, DC - 1)
                qk_mm(False, 1, DC - 1)
                nc.scalar.copy(kT_lo, kps_lo)
                qk_mm(True, 2, DC - 1)
                qk_mm(True, 3, DC - 1)
                nc.vector.tensor_copy(qT_sb[:, 1024:2048], qps_hi)
                qk_mm(False, 2, DC - 1)
                qk_mm(False, 3, DC - 1)
                nc.scalar.copy(kT_hi, kps_hi)

            def kT_block(j):
                return (kT_lo if j < 8 else kT_hi)[:, (j % 8) * P : (j % 8 + 1) * P]

            with ExitStack() as ctx2:
                psS = ctx2.enter_context(
                    tc.tile_pool(name="psS", bufs=2, space="PSUM")
                )
                psV = ctx2.enter_context(
                    tc.tile_pool(name="psV", bufs=1, space="PSUM")
                )
                psO = ctx2.enter_context(
                    tc.tile_pool(name="psO", bufs=2, space="PSUM")
                )
                psO15 = ctx2.enter_context(
                    tc.tile_pool(name="psO15", bufs=1, space="PSUM")
                )

                # ones column of v_aug, once (region disjoint from v copies)
                nc.vector.memset(v_sb[:, :, 128:129], 1.0)

                out_ap = out_h[:]

                # Causal-packed wT layout: segment for k-block j holds
                # q in [j*128, T) at packed offset OFF[j].
                OFF = [0] * (TB + 1)
                for j in range(TB):
                    OFF[j + 1] = OFF[j] + (T - j * P)
                TOTAL = OFF[TB]  # 17408

                def wT_at(jj, qstart, width):
                    o = OFF[jj] + (qstart - jj * P)
                    return wT_sb[:, o : o + width]

                def emit_v(j):
                    pv = psV.tile([P, H], f32, tag="psV")
                    for c in range(DC):
                        nc.tensor.matmul(
                            pv,
                            xT_sb[:, c, j * P : (j + 1) * P],
                            wv_sb[:, c, :],
                            start=(c == 0),
                            stop=(c == DC - 1),
                        )
                    nc.vector.tensor_copy(v_sb[:, j, 0:H], pv)

                def emit_epilogue(i, po):
                    nc.vector.reciprocal(rec_all[:, i : i + 1], po[:, 128:129])
                    m = nc.scalar.mul(
                        ot_all[:, i, :], po[:, 0:H], rec_all[:, i : i + 1]
                    )
                    mul_names.append(m.ins.name)
                    for a, b in ((0, 4), (4, 8), (8, 12), (12, 16)):
                        if i == b - 1:
                            # rearrange so DMA iteration order (p, blk, h) on
                            # the SBUF side pairs with row blk*128+p in DRAM
                            nc.sync.dma_start(
                                out=out_ap[a * P : b * P, :].rearrange(
                                    "(b p) h -> p b h", p=P
                                ),
                                in_=ot_all[:, a:b, :],
                            )

                def emit_c_group(i):
                    po = psO.tile([P, 132], f32, tag="psO", name=f"po{i}")
                    for jj in range(i):
                        nc.tensor.matmul(
                            po[:, 0:129],
                            wT_at(jj, i * P, P),
                            v_sb[:, jj, 0:129],
                            start=(jj == 0),
                            stop=False,
                        )
                    nc.tensor.matmul(
                        po[:, 0:129],
                        dw_sb[:, i, :],
                        v_sb[:, i, 0:129],
                        start=(i == 0),
                        stop=True,
                    )
                    emit_epilogue(i, po)

                # exp pieces: 16x1024, then the last 1024 split at segment
                # boundaries (640 | 256 | 128) so late epilogues fire early.
                pieces = [(p * 1024, 1024) for p in range(16)]
                pieces += [(16384, 640), (17024, 256), (17280, 128)]

                po15 = psO15.tile([P, 132], f32, tag="po15")
                next15 = 0  # next C(15) k-block whose matmul is pending
                pending_c = None
                next_done = 0  # next j whose dw/epilogue trigger is pending

                # v_0/v_1 ahead of everything: zero-dep PE filler covering
                # the qT/kT eviction latency at the A1->B boundary.
                emit_v(0)
                emit_v(1)

                for pi, (ts, tw) in enumerate(pieces):
                    ps = psS.tile([P, 1024], f32, tag="psS")
                    # score matmuls covering packed [ts, ts+tw): split at
                    # PSUM bank boundaries and segment boundaries.
                    for j in range(TB):
                        lo = max(ts, OFF[j])
                        hi = min(ts + tw, OFF[j + 1])
                        a = lo
                        while a < hi:
                            bank_end = ts + ((a - ts) // 512 + 1) * 512
                            b = min(hi, bank_end)
                            qg = j * P + (a - OFF[j])
                            nc.tensor.matmul(
                                ps[:, a - ts : b - ts],
                                kT_block(j),
                                qT_sb[:, qg : qg + (b - a)],
                                start=True,
                                stop=True,
                            )
                            a = b
                    # C(15) accumulation: k-blocks whose segment is fully
                    # exp'd (emitted pre-exp so the whole-tile wT wait
                    # resolves to the previous piece's exp).
                    while next15 < TB - 1 and OFF[next15 + 1] <= ts:
                        nc.tensor.matmul(
                            po15[:, 0:129],
                            wT_at(next15, (TB - 1) * P, P),
                            v_sb[:, next15, 0:129],
                            start=(next15 == 0),
                            stop=False,
                        )
                        next15 += 1
                    # delayed C group: its wT dependency is the PREVIOUS exp
                    if pending_c is not None:
                        emit_c_group(pending_c)
                        pending_c = None
                    nc.scalar.activation(
                        wT_sb[:, ts : ts + tw], ps[:, :tw], EXP, scale=SCALE
                    )
                    if pi <= 13:
                        emit_v(pi + 2)
                    # epilogue trigger j: segment j's first 128 cols exp'd
                    while next_done < TB and OFF[next_done] + P <= ts + tw:
                        j = next_done
                        nc.vector.tensor_tensor(
                            dw_sb[:, j, :], wT_at(j, j * P, P), mask2_sb, MULT
                        )
                        if j > 0:
                            if pending_c is not None:
                                emit_c_group(pending_c)
                            pending_c = j - 1
                        next_done += 1

                # tail: C(14), then C(15)'s last off-diag + masked diagonal
                if pending_c is not None:
                    emit_c_group(pending_c)  # C(14)
                i15 = TB - 1
                while next15 < TB - 1:
                    nc.tensor.matmul(
                        po15[:, 0:129],
                        wT_at(next15, i15 * P, P),
                        v_sb[:, next15, 0:129],
                        start=(next15 == 0),
                        stop=False,
                    )
                    next15 += 1
                nc.tensor.matmul(
                    po15[:, 0:129],
                    dw_sb[:, i15, :],
                    v_sb[:, i15, 0:129],
                    start=False,
                    stop=True,
                )
                emit_epilogue(i15, po15)

    _strip_self_waits(nc)
    _strip_mul_pe_waits(nc, mul_names)
    nc.finalize()  # Bacc.compile(): wait legalization + register allocation
    return nc


def _strip_self_waits(nc):
    """Drop same-engine semaphore waits on in-order engines (PE/ACT/DVE
    execute and complete strictly in order, so a self-wait is redundant).
    Tile emits them conservatively; walrus allows only one sem wait per
    compute instruction, and these push some matmuls/tensor-ops over."""
    prefixes = {"PE": "PE_", "Activation": "Activation_", "DVE": "DVE_"}
    for bb in nc.m.functions[0].blocks:
        for inst in bb.instructions:
            si = inst.sync_info
            if not si or not si.on_wait:
                continue
            pref = prefixes.get(str(inst.engine).split(".")[-1])
            if pref is None:
                continue
            keep = [w for w in si.on_wait if not (w.ant_name or "").startswith(pref)]
            if len(keep) != len(si.on_wait):
                si.on_wait = keep
                inst.sync_info = si


def _strip_mul_pe_waits(nc, mul_names):
    """The out-scale mul waits {PE(po stop-matmul), DVE(reciprocal)}; the
    reciprocal itself waits on that same PE stop-matmul, so the mul's PE
    wait is transitively implied by its DVE wait. Dropping it leaves one
    wait -> no event-semaphore junction (each junction costs ~115ns per
    engine in the NEFF teardown sweep)."""
    names = set(mul_names)
    for bb in nc.m.functions[0].blocks:
        for inst in bb.instructions:
            if getattr(inst, "name", None) not in names:
                continue
            si = inst.sync_info
            if not si or not si.on_wait:
                continue
            keep = [w for w in si.on_wait if (w.ant_name or "").startswith("DVE")]
            if keep and len(keep) != len(si.on_wait):
                si.on_wait = keep
                inst.sync_info = si


def kernel(**inputs):
    global LAST_RESULT
    x = np.asarray(inputs["x"], dtype=np.float32)
    bf = ml_dtypes.bfloat16
    w_bf = {
        k: np.ascontiguousarray(
            np.asarray(inputs[k], dtype=np.float32)
            .astype(bf)
            .reshape(DC, P, H)
            .transpose(1, 0, 2)
        )
        for k in ("Wq", "Wk", "Wv")
    }
    # dw[p=k_local, f=q_local] keeps entries with k <= q
    mask01 = (
        (np.arange(P)[:, None] <= np.arange(P)[None, :]).astype(np.float32).astype(bf)
    )

    if "nc" not in _CACHE:
        _CACHE["nc"] = _build()
    nc = _CACHE["nc"]

    from concourse.bass_utils import run_bass_kernel_spmd

    in_maps = [
        {
            "xT": np.ascontiguousarray(x[b].T).astype(bf),
            "Wq": w_bf["Wq"],
            "Wk": w_bf["Wk"],
            "Wv": w_bf["Wv"],
            "mask": mask01,
        }
        for b in range(B)
    ]
    res = run_bass_kernel_spmd(nc, in_maps, core_ids=list(range(B)))
    LAST_RESULT = res
    return np.stack([res.results[b]["out"] for b in range(B)]).astype(np.float32)


# revision 13
# speedup vs baseline: 1.0369x; 1.0369x over previous
"""Trainium2 (8 NeuronCores) kernel for single-head causal attention.

Problem: x [8, 2048, 1024] f32; Wq/Wk/Wv [1024, 128] f32.
    q = x @ Wq ; k = x @ Wk ; v = x @ Wv          (per batch row)
    out = softmax(causal(q @ k^T / sqrt(128))) @ v  -> [8, 2048, 128] f32
Sharding: pure data-parallel - one batch row per NeuronCore, weights
replicated. No collectives.

Per-core algorithm (bf16 matmul inputs, f32 PSUM accumulation):
  Host supplies xT = x[b].T  [D, T] in bf16 (layout prep only).
  A) qT [H=128 part, T] / kT (split lo|hi tiles) with W-chunks stationary
     over 8 D-chunks, d-outer so PE tracks the xT DMA chunk by chunk.
     wq rides the SYNC ring ahead of the 8 xT chunk FIFO DMAs; wk/wv/mask
     ride the ACT ring - so neither weight gates the first chunk's q AND k
     matmuls (phase A is input-bandwidth-bound; order is everything).
  B) Scores TRANSPOSED: sT[k,q] = kT_j-block stationary @ qT, exact-causal,
     exp(scale*s) on ScalarE from PSUM into a CAUSAL-PACKED bf16 wT tile
     (17 maximal 1024-wide pieces, the last 1024 split 640/256/128 at
     segment boundaries so the final row-blocks' epilogues start early).
     No max-subtraction: |scale*s| <= ~7, safe in f32/bf16. Diagonal
     blocks get a multiplicative 0/1 strictly-causal mask into dw tiles
     on DVE (off the PE critical path).
  C) out[q,h] accumulates k-blocks jj<=i with wT stationary / v_aug moving
     (N=129; col 128 = ones => softmax denominator falls out free).
     C-group emission is DELAYED one piece: a group is emitted before the
     NEXT piece's exp, so its conservative whole-wT-tile dependency
     resolves to the PREVIOUS exp - already complete - and the PE stream
     never stalls on ScalarE. v-projections are spread one per piece as
     zero-dependency PE filler. C(15) is special-cased: its jj-th matmul
     is emitted as soon as segment jj is fully exp'd, so after the last
     (128-wide) exp only its diagonal matmul + epilogue remain.
     Normalize: DVE reciprocal + ScalarE copy-with-per-partition-scale
     (DVE tensor_scalar AP / stride-0 broadcasts are wrong on HW; ScalarE
     Reciprocal would thrash the activation table against Exp).
     Output DMAs are GROUPED {4,4,4,2,1,1} on the sync ring: fewer ring-
     slot-credit waits -> fewer event semaphores. The NEFF teardown clears
     every event semaphore at ~115ns each on every engine, so junction
     count is a first-class cost (~6.5us of the baseline's tail).

Engine discipline: hardware compute instructions carry at most ONE
semaphore wait; bacc legalizes extras into event-semaphore junctions
(move_matmul_waits_to_ldweights gives PE pairs two slots). Tile tracks
dependencies at TILE granularity. Post-build passes: (1) strip redundant
same-engine self-waits (PE/ACT/DVE complete in order); (2) strip the
out-scale mul's PE wait - it is transitively implied by its DVE wait
(the reciprocal it consumes already waited on the same PSUM stop-matmul).
"""

from contextlib import ExitStack

import ml_dtypes
import numpy as np

B, T, D, H = 8, 2048, 1024, 128
P = 128
DC = D // P  # 8 contraction chunks
TB = T // P  # 16 token blocks
QG = T // 512  # 4 512-wide token groups
SCALE = 1.0 / float(np.sqrt(H))

_CACHE = {}
LAST_RESULT = None


def _build():
    import concourse.bacc as bacc
    import concourse.mybir as mybir
    import concourse.tile as tile

    f32 = mybir.dt.float32
    bf16 = mybir.dt.bfloat16
    EXP = mybir.ActivationFunctionType.Exp
    MULT = mybir.AluOpType.mult

    nc = bacc.Bacc()
    # host-prelayouted so every 256KB half-chunk (c, h) is CONTIGUOUS in
    # DRAM (row r = c*256 + h*128 + p, cols = that half's 1024 tokens)
    xT_h = nc.declare_dram_parameter("xT", [2 * D, T // 2], bf16, isOutput=False)
    # weights host-prelayouted to [p, c, h]: contiguous 2048 B partition rows
    wq_h = nc.declare_dram_parameter("Wq", [P, DC, H], bf16, isOutput=False)
    wk_h = nc.declare_dram_parameter("Wk", [P, DC, H], bf16, isOutput=False)
    wv_h = nc.declare_dram_parameter("Wv", [P, DC, H], bf16, isOutput=False)
    mask_h = nc.declare_dram_parameter("mask", [P, P], bf16, isOutput=False)
    out_h = nc.declare_dram_parameter("out", [T, H], f32, isOutput=True)

    mul_names = []  # out-scale muls whose PE wait is transitively implied

    with tile.TileContext(nc) as tc:
        with ExitStack() as ctx:
            singles = ctx.enter_context(tc.tile_pool(name="singles", bufs=1))

            xT_sb = singles.tile([P, DC, T], bf16)
            wq_sb = singles.tile([P, DC, H], bf16)
            wk_sb = singles.tile([P, DC, H], bf16)
            wv_sb = singles.tile([P, DC, H], bf16)
            mask_sb = singles.tile([P, P], bf16)
            mask2_sb = singles.tile([P, P], bf16)
            qT_sb = singles.tile([P, T], bf16)
            kT_lo = singles.tile([P, T // 2], bf16)  # k-blocks 0..7
            kT_hi = singles.tile([P, T // 2], bf16)  # k-blocks 8..15
            v_sb = singles.tile([P, TB, 132], bf16)  # [...,128] = ones col
            wT_sb = singles.tile([P, 17408], bf16)  # causal-packed
            dw_sb = singles.tile([P, TB, P], bf16)  # masked diagonal blocks
            rec_all = singles.tile([P, TB], f32)
            ot_all = singles.tile([P, TB, H], f32)

            # SYNC ring: wq first (A1's q-side gate), then xT as 16 HALF-
            # chunk FIFO DMAs - same-queue DMAs stream sequentially so the
            # first half lands ~1.5us after wq and the d-outer projection
            # loop tracks the input at 256KB granularity; the last unit of
            # PE work gated on the final DMA byte is one 1024-col q/k pair
            # (~0.9us) instead of a whole chunk.
            nc.sync.dma_start(out=wq_sb, in_=wq_h[:])
            xT_ap = xT_h[:]
            for c in range(DC):
                for h in (0, 1):
                    r = (2 * c + h) * P
                    nc.sync.dma_start(
                        out=xT_sb[:, c, h * 1024 : (h + 1) * 1024],
                        in_=xT_ap[r : r + P, :],
                    )
            nc.scalar.dma_start(out=wk_sb, in_=wk_h[:])
            nc.scalar.dma_start(out=wv_sb, in_=wv_h[:])
            nc.scalar.dma_start(out=mask_sb, in_=mask_h[:])
            # ACT pre-touch: moves the mask's DMA wait onto a junction copy
            # so the per-j diag multiply's two deps merge into one ACT wait.
            nc.scalar.copy(mask2_sb, mask_sb)

            # PE warm-up fodder: HAM starts the PE throttled at 1.2 GHz and
            # needs ~3.4 us of sustained work to unthrottle; these dummies
            # run in the launch/DMA dead window. They write qps[0] BEFORE
            # its real accumulation group begins (start=True clears it).
            warm_sb = singles.tile([P, 512], bf16)
            nc.vector.memset(warm_sb, 0.0)

            # --- Phase A1: q/k projections, d-chunk OUTER so each xT
            # half-chunk is consumed as its DMA lands. qps/kps are SPLIT
            # lo|hi (2 banks each) so the first score piece's PSUM WAR
            # resolves against the EARLIEST eviction copy, not the last.
            with tc.tile_pool(name="psQK", bufs=1, space="PSUM") as psQK:
                qps_lo = psQK.tile([P, 1024], f32, tag="qps_lo")
                qps_hi = psQK.tile([P, 1024], f32, tag="qps_hi")
                kps_lo = psQK.tile([P, 1024], f32, tag="kps_lo")
                kps_hi = psQK.tile([P, 1024], f32, tag="kps_hi")
                for _ in range(8):
                    nc.tensor.matmul(
                        qps_lo[:, 0:512], warm_sb[:, 0:128], warm_sb,
                        start=True, stop=True,
                    )

                def qk_mm(is_q, g, c):
                    w_sb = wq_sb if is_q else wk_sb
                    acc = (
                        (qps_lo if g < 2 else qps_hi)
                        if is_q
                        else (kps_lo if g < 2 else kps_hi)
                    )
                    nc.tensor.matmul(
                        acc[:, (g % 2) * 512 : (g % 2 + 1) * 512],
                        w_sb[:, c, :],
                        xT_sb[:, c, g * 512 : (g + 1) * 512],
                        start=(c == 0),
                        stop=(c == DC - 1),
                    )

                for c in range(DC - 1):
                    for is_q in (True, False):
                        for g in (0, 1):  # half 0 of chunk c
                            qk_mm(is_q, g, c)
                    for is_q in (True, False):
                        for g in (2, 3):  # half 1 of chunk c
                            qk_mm(is_q, g, c)
                # Last chunk: evictions launched per half as it completes,
                # q-lo first (DVE) then k-lo (ACT) so the two copy engines
                # pipeline and the first score piece's gates (qT-lo cast,
                # kT-lo copy, qps-lo WAR) all resolve early.
                qk_mm(True, 0, DC - 1)
                qk_mm(True, 1, DC - 1)
                nc.vector.tensor_copy(qT_sb[:, 0:1024], qps_lo)
                qk_mm(False, 0, DC - 1)
                qk_mm(False, 1, DC - 1)
                nc.scalar.copy(kT_lo, kps_lo)
                qk_mm(True, 2, DC - 1)
                qk_mm(True, 3, DC - 1)
                nc.vector.tensor_copy(qT_sb[:, 1024:2048], qps_hi)
                qk_mm(False, 2, DC - 1)
                qk_mm(False, 3, DC - 1)
                nc.scalar.copy(kT_hi, kps_hi)

            def kT_block(j):
                return (kT_lo if j < 8 else kT_hi)[:, (j % 8) * P : (j % 8 + 1) * P]

            with ExitStack() as ctx2:
                psS = ctx2.enter_context(
                    tc.tile_pool(name="psS", bufs=2, space="PSUM")
                )
                psV = ctx2.enter_context(
                    tc.tile_pool(name="psV", bufs=1, space="PSUM")
                )
                psO = ctx2.enter_context(
                    tc.tile_pool(name="psO", bufs=2, space="PSUM")
                )
                psO15 = ctx2.enter_context(
                    tc.tile_pool(name="psO15", bufs=1, space="PSUM")
                )

                # ones column of v_aug, once (region disjoint from v copies)
                nc.vector.memset(v_sb[:, :, 128:129], 1.0)

                out_ap = out_h[:]

                # Causal-packed wT layout: segment for k-block j holds
                # q in [j*128, T) at packed offset OFF[j].
                OFF = [0] * (TB + 1)
                for j in range(TB):
                    OFF[j + 1] = OFF[j] + (T - j * P)
                TOTAL = OFF[TB]  # 17408

                def wT_at(jj, qstart, width):
                    o = OFF[jj] + (qstart - jj * P)
                    return wT_sb[:, o : o + width]

                def emit_v(j):
                    pv = psV.tile([P, H], f32, tag="psV")
                    for c in range(DC):
                        nc.tensor.matmul(
                            pv,
                            xT_sb[:, c, j * P : (j + 1) * P],
                            wv_sb[:, c, :],
                            start=(c == 0),
                            stop=(c == DC - 1),
                        )
                    nc.vector.tensor_copy(v_sb[:, j, 0:H], pv)

                def emit_epilogue(i, po):
                    nc.vector.reciprocal(rec_all[:, i : i + 1], po[:, 128:129])
                    m = nc.scalar.mul(
                        ot_all[:, i, :], po[:, 0:H], rec_all[:, i : i + 1]
                    )
                    mul_names.append(m.ins.name)
                    for a, b in ((0, 4), (4, 8), (8, 12), (12, 16)):
                        if i == b - 1:
                            # rearrange so DMA iteration order (p, blk, h) on
                            # the SBUF side pairs with row blk*128+p in DRAM
                            nc.sync.dma_start(
                                out=out_ap[a * P : b * P, :].rearrange(
                                    "(b p) h -> p b h", p=P
                                ),
                                in_=ot_all[:, a:b, :],
                            )

                def emit_c_group(i):
                    po = psO.tile([P, 132], f32, tag="psO", name=f"po{i}")
                    for jj in range(i):
                        nc.tensor.matmul(
                            po[:, 0:129],
                            wT_at(jj, i * P, P),
                            v_sb[:, jj, 0:129],
                            start=(jj == 0),
                            stop=False,
                        )
                    nc.tensor.matmul(
                        po[:, 0:129],
                        dw_sb[:, i, :],
                        v_sb[:, i, 0:129],
                        start=(i == 0),
                        stop=True,
                    )
                    emit_epilogue(i, po)

                # exp pieces: 16x1024, then the last 1024 split at segment
                # boundaries (640 | 256 | 128) so late epilogues fire early.
                pieces = [(p * 1024, 1024) for p in range(16)]
                pieces += [(16384, 640), (17024, 256), (17280, 128)]

                po15 = psO15.tile([P, 132], f32, tag="po15")
                next15 = 0  # next C(15) k-block whose matmul is pending
                pending_c = None
                next_done = 0  # next j whose dw/epilogue trigger is pending

                # v_0/v_1 ahead of everything: zero-dep PE filler covering
                # the qT/kT eviction latency at the A1->B boundary.
                emit_v(0)
                emit_v(1)

                for pi, (ts, tw) in enumerate(pieces):
                    ps = psS.tile([P, 1024], f32, tag="psS")
                    # score matmuls covering packed [ts, ts+tw): split at
                    # PSUM bank boundaries and segment boundaries.
                    for j in range(TB):
                        lo = max(ts, OFF[j])
                        hi = min(ts + tw, OFF[j + 1])
                        a = lo
                        while a < hi:
                            bank_end = ts + ((a - ts) // 512 + 1) * 512
                            b = min(hi, bank_end)
                            qg = j * P + (a - OFF[j])
                            nc.tensor.matmul(
                                ps[:, a - ts : b - ts],
                                kT_block(j),
                                qT_sb[:, qg : qg + (b - a)],
                                start=True,
                                stop=True,
                            )
                            a = b
                    # C(15) accumulation: k-blocks whose segment is fully
                    # exp'd (emitted pre-exp so the whole-tile wT wait
                    # resolves to the previous piece's exp).
                    while next15 < TB - 1 and OFF[next15 + 1] <= ts:
                        nc.tensor.matmul(
                            po15[:, 0:129],
                            wT_at(next15, (TB - 1) * P, P),
                            v_sb[:, next15, 0:129],
                            start=(next15 == 0),
                            stop=False,
                        )
                        next15 += 1
                    # delayed C group: its wT dependency is the PREVIOUS exp
                    if pending_c is not None:
                        emit_c_group(pending_c)
                        pending_c = None
                    nc.scalar.activation(
                        wT_sb[:, ts : ts + tw], ps[:, :tw], EXP, scale=SCALE
                    )
                    if pi <= 13:
                        emit_v(pi + 2)
                    # epilogue trigger j: segment j's first 128 cols exp'd
                    while next_done < TB and OFF[next_done] + P <= ts + tw:
                        j = next_done
                        nc.vector.tensor_tensor(
                            dw_sb[:, j, :], wT_at(j, j * P, P), mask2_sb, MULT
                        )
                        if j > 0:
                            if pending_c is not None:
                                emit_c_group(pending_c)
                            pending_c = j - 1
                        next_done += 1

                # tail: C(14), then C(15)'s last off-diag + masked diagonal
                if pending_c is not None:
                    emit_c_group(pending_c)  # C(14)
                i15 = TB - 1
                while next15 < TB - 1:
                    nc.tensor.matmul(
                        po15[:, 0:129],
                        wT_at(next15, i15 * P, P),
                        v_sb[:, next15, 0:129],
                        start=(next15 == 0),
                        stop=False,
                    )
                    next15 += 1
                nc.tensor.matmul(
                    po15[:, 0:129],
                    dw_sb[:, i15, :],
                    v_sb[:, i15, 0:129],
                    start=False,
                    stop=True,
                )
                emit_epilogue(i15, po15)

    _strip_self_waits(nc)
    _strip_mul_pe_waits(nc, mul_names)
    nc.finalize()  # Bacc.compile(): wait legalization + register allocation
    return nc


def _strip_self_waits(nc):
    """Drop same-engine semaphore waits on in-order engines (PE/ACT/DVE
    execute and complete strictly in order, so a self-wait is redundant).
    Tile emits them conservatively; walrus allows only one sem wait per
    compute instruction, and these push some matmuls/tensor-ops over."""
    prefixes = {"PE": "PE_", "Activation": "Activation_", "DVE": "DVE_"}
    for bb in nc.m.functions[0].blocks:
        for inst in bb.instructions:
            si = inst.sync_info
            if not si or not si.on_wait:
                continue
            pref = prefixes.get(str(inst.engine).split(".")[-1])
            if pref is None:
                continue
            keep = [w for w in si.on_wait if not (w.ant_name or "").startswith(pref)]
            if len(keep) != len(si.on_wait):
                si.on_wait = keep
                inst.sync_info = si


def _strip_mul_pe_waits(nc, mul_names):
    """The out-scale mul waits {PE(po stop-matmul), DVE(reciprocal)}; the
    reciprocal itself waits on that same PE stop-matmul, so the mul's PE
    wait is transitively implied by its DVE wait. Dropping it leaves one
    wait -> no event-semaphore junction (each junction costs ~115ns per
    engine in the NEFF teardown sweep)."""
    names = set(mul_names)
    for bb in nc.m.functions[0].blocks:
        for inst in bb.instructions:
            if getattr(inst, "name", None) not in names:
                continue
            si = inst.sync_info
            if not si or not si.on_wait:
                continue
            keep = [w for w in si.on_wait if (w.ant_name or "").startswith("DVE")]
            if keep and len(keep) != len(si.on_wait):
                si.on_wait = keep
                inst.sync_info = si


def kernel(**inputs):
    global LAST_RESULT
    x = np.asarray(inputs["x"], dtype=np.float32)
    bf = ml_dtypes.bfloat16
    w_bf = {
        k: np.ascontiguousarray(
            np.asarray(inputs[k], dtype=np.float32)
            .astype(bf)
            .reshape(DC, P, H)
            .transpose(1, 0, 2)
        )
        for k in ("Wq", "Wk", "Wv")
    }
    # dw[p=k_local, f=q_local] keeps entries with k <= q
    mask01 = (
        (np.arange(P)[:, None] <= np.arange(P)[None, :]).astype(np.float32).astype(bf)
    )

    if "nc" not in _CACHE:
        _CACHE["nc"] = _build()
    nc = _CACHE["nc"]

    from concourse.bass_utils import run_bass_kernel_spmd

    in_maps = [
        {
            "xT": np.ascontiguousarray(
                x[b].T.reshape(DC, P, 2, T // 2).transpose(0, 2, 1, 3)
            )
            .reshape(2 * D, T // 2)
            .astype(bf),
            "Wq": w_bf["Wq"],
            "Wk": w_bf["Wk"],
            "Wv": w_bf["Wv"],
            "mask": mask01,
        }
        for b in range(B)
    ]
    res = run_bass_kernel_spmd(nc, in_maps, core_ids=list(range(B)))
    LAST_RESULT = res
    return np.stack([res.results[b]["out"] for b in range(B)]).astype(np.float32)


# revision 20
# speedup vs baseline: 1.0404x; 1.0034x over previous
"""Trainium2 (8 NeuronCores) kernel for single-head causal attention.

Problem: x [8, 2048, 1024] f32; Wq/Wk/Wv [1024, 128] f32.
    q = x @ Wq ; k = x @ Wk ; v = x @ Wv          (per batch row)
    out = softmax(causal(q @ k^T / sqrt(128))) @ v  -> [8, 2048, 128] f32
Sharding: pure data-parallel - one batch row per NeuronCore, weights
replicated. No collectives.

Per-core algorithm (bf16 matmul inputs, f32 PSUM accumulation):
  Host supplies xT = x[b].T  [D, T] in bf16 (layout prep only).
  A) qT [H=128 part, T] / kT (split lo|hi tiles) with W-chunks stationary
     over 8 D-chunks, d-outer so PE tracks the xT DMA chunk by chunk.
     wq rides the SYNC ring ahead of the 8 xT chunk FIFO DMAs; wk/wv/mask
     ride the ACT ring - so neither weight gates the first chunk's q AND k
     matmuls (phase A is input-bandwidth-bound; order is everything).
  B) Scores TRANSPOSED: sT[k,q] = kT_j-block stationary @ qT, exact-causal,
     exp(scale*s) on ScalarE from PSUM into a CAUSAL-PACKED bf16 wT tile
     (17 maximal 1024-wide pieces, the last 1024 split 640/256/128 at
     segment boundaries so the final row-blocks' epilogues start early).
     No max-subtraction: |scale*s| <= ~7, safe in f32/bf16. Diagonal
     blocks get a multiplicative 0/1 strictly-causal mask into dw tiles
     on DVE (off the PE critical path).
  C) out[q,h] accumulates k-blocks jj<=i with wT stationary / v_aug moving
     (N=129; col 128 = ones => softmax denominator falls out free).
     C-group emission is DELAYED one piece: a group is emitted before the
     NEXT piece's exp, so its conservative whole-wT-tile dependency
     resolves to the PREVIOUS exp - already complete - and the PE stream
     never stalls on ScalarE. v-projections are spread one per piece as
     zero-dependency PE filler. C(15) is special-cased: its jj-th matmul
     is emitted as soon as segment jj is fully exp'd, so after the last
     (128-wide) exp only its diagonal matmul + epilogue remain.
     Normalize: DVE reciprocal + ScalarE copy-with-per-partition-scale
     (DVE tensor_scalar AP / stride-0 broadcasts are wrong on HW; ScalarE
     Reciprocal would thrash the activation table against Exp).
     Output DMAs are GROUPED {4,4,4,2,1,1} on the sync ring: fewer ring-
     slot-credit waits -> fewer event semaphores. The NEFF teardown clears
     every event semaphore at ~115ns each on every engine, so junction
     count is a first-class cost (~6.5us of the baseline's tail).

Engine discipline: hardware compute instructions carry at most ONE
semaphore wait; bacc legalizes extras into event-semaphore junctions
(move_matmul_waits_to_ldweights gives PE pairs two slots). Tile tracks
dependencies at TILE granularity. Post-build passes: (1) strip redundant
same-engine self-waits (PE/ACT/DVE complete in order); (2) strip the
out-scale mul's PE wait - it is transitively implied by its DVE wait
(the reciprocal it consumes already waited on the same PSUM stop-matmul).
"""

from contextlib import ExitStack

import ml_dtypes
import numpy as np

B, T, D, H = 8, 2048, 1024, 128
P = 128
DC = D // P  # 8 contraction chunks
TB = T // P  # 16 token blocks
QG = T // 512  # 4 512-wide token groups
SCALE = 1.0 / float(np.sqrt(H))

_CACHE = {}
LAST_RESULT = None


def _build():
    import concourse.bacc as bacc
    import concourse.mybir as mybir
    import concourse.tile as tile

    f32 = mybir.dt.float32
    bf16 = mybir.dt.bfloat16
    EXP = mybir.ActivationFunctionType.Exp
    MULT = mybir.AluOpType.mult

    nc = bacc.Bacc()
    xT_h = nc.declare_dram_parameter("xT", [D, T], bf16, isOutput=False)
    # weights host-prelayouted to [p, c, h]: contiguous 2048 B partition rows
    wq_h = nc.declare_dram_parameter("Wq", [P, DC, H], bf16, isOutput=False)
    wk_h = nc.declare_dram_parameter("Wk", [P, DC, H], bf16, isOutput=False)
    wv_h = nc.declare_dram_parameter("Wv", [P, DC, H], bf16, isOutput=False)
    mask_h = nc.declare_dram_parameter("mask", [P, P], bf16, isOutput=False)
    out_h = nc.declare_dram_parameter("out", [T, H], f32, isOutput=True)

    mul_names = []  # out-scale muls whose PE wait is transitively implied

    with tile.TileContext(nc) as tc:
        with ExitStack() as ctx:
            singles = ctx.enter_context(tc.tile_pool(name="singles", bufs=1))

            xT_sb = singles.tile([P, DC, T], bf16)
            wq_sb = singles.tile([P, DC, H], bf16)
            wk_sb = singles.tile([P, DC, H], bf16)
            wv_sb = singles.tile([P, DC, H], bf16)
            mask_sb = singles.tile([P, P], bf16)
            mask2_sb = singles.tile([P, P], bf16)
            qT_lo = singles.tile([P, T // 2], bf16)  # q in [0, 1024)
            qT_hi = singles.tile([P, T // 2], bf16)  # q in [1024, 2048)
            kT_lo = singles.tile([P, T // 2], bf16)  # k-blocks 0..7
            kT_hi = singles.tile([P, T // 2], bf16)  # k-blocks 8..15
            v_sb = singles.tile([P, TB, 132], bf16)  # [...,128] = ones col
            wT_sb = singles.tile([P, 17408], bf16)  # causal-packed
            dw_sb = singles.tile([P, TB, P], bf16)  # masked diagonal blocks
            rec_all = singles.tile([P, TB], f32)
            ot_all = singles.tile([P, TB, H], f32)

            # SYNC ring: wq first (A1's q-side gate), then the 8 xT chunks
            # as FIFO DMAs - same-queue DMAs stream sequentially so chunk 0
            # completes right after wq and the d-outer projection loop
            # tracks the input as it lands. Full 512KB contiguous chunks:
            # finer splits lower the DMA's effective HBM bandwidth.
            nc.sync.dma_start(out=wq_sb, in_=wq_h[:])
            xT_ap = xT_h[:]
            for c in range(DC):
                nc.sync.dma_start(
                    out=xT_sb[:, c, :], in_=xT_ap[c * P : (c + 1) * P, :]
                )
            nc.scalar.dma_start(out=wk_sb, in_=wk_h[:])
            nc.scalar.dma_start(out=wv_sb, in_=wv_h[:])
            nc.scalar.dma_start(out=mask_sb, in_=mask_h[:])
            # ACT pre-touch: moves the mask's DMA wait onto a junction copy
            # so the per-j diag multiply's two deps merge into one ACT wait.
            nc.scalar.copy(mask2_sb, mask_sb)

            # PE warm-up fodder: HAM starts the PE throttled at 1.2 GHz and
            # needs ~3.4 us of sustained work to unthrottle; these dummies
            # run in the launch/DMA dead window. They write qps[0] BEFORE
            # its real accumulation group begins (start=True clears it).
            warm_sb = singles.tile([P, 512], bf16)
            nc.vector.memset(warm_sb, 0.0)

            # --- Phase A1: q/k projections, d-chunk OUTER so each xT
            # half-chunk is consumed as its DMA lands. qps/kps are SPLIT
            # lo|hi (2 banks each) so the first score piece's PSUM WAR
            # resolves against the EARLIEST eviction copy, not the last.
            with tc.tile_pool(name="psQK", bufs=1, space="PSUM") as psQK:
                qps_lo = psQK.tile([P, 1024], f32, tag="qps_lo")
                qps_hi = psQK.tile([P, 1024], f32, tag="qps_hi")
                kps_lo = psQK.tile([P, 1024], f32, tag="kps_lo")
                kps_hi = psQK.tile([P, 1024], f32, tag="kps_hi")
                for _ in range(12):
                    nc.tensor.matmul(
                        qps_lo[:, 0:512], warm_sb[:, 0:128], warm_sb,
                        start=True, stop=True,
                    )

                def qk_mm(is_q, g, c):
                    w_sb = wq_sb if is_q else wk_sb
                    acc = (
                        (qps_lo if g < 2 else qps_hi)
                        if is_q
                        else (kps_lo if g < 2 else kps_hi)
                    )
                    nc.tensor.matmul(
                        acc[:, (g % 2) * 512 : (g % 2 + 1) * 512],
                        w_sb[:, c, :],
                        xT_sb[:, c, g * 512 : (g + 1) * 512],
                        start=(c == 0),
                        stop=(c == DC - 1),
                    )

                for c in range(DC - 1):
                    for is_q in (True, False):
                        for g in (0, 1):  # half 0 of chunk c
                            qk_mm(is_q, g, c)
                    for is_q in (True, False):
                        for g in (2, 3):  # half 1 of chunk c
                            qk_mm(is_q, g, c)
                # Last chunk: evictions launched per half as it completes,
                # q-lo first (DVE) then k-lo (ACT) so the two copy engines
                # pipeline and the first score piece's gates (qT-lo cast,
                # kT-lo copy, qps-lo WAR) all resolve early.
                qk_mm(True, 0, DC - 1)
                qk_mm(True, 1, DC - 1)
                nc.vector.tensor_copy(qT_lo, qps_lo)
                qk_mm(False, 0, DC - 1)
                qk_mm(False, 1, DC - 1)
                nc.scalar.copy(kT_lo, kps_lo)
                qk_mm(True, 2, DC - 1)
                qk_mm(True, 3, DC - 1)
                nc.vector.tensor_copy(qT_hi, qps_hi)
                qk_mm(False, 2, DC - 1)
                qk_mm(False, 3, DC - 1)
                nc.scalar.copy(kT_hi, kps_hi)

            def kT_block(j):
                return (kT_lo if j < 8 else kT_hi)[:, (j % 8) * P : (j % 8 + 1) * P]

            with ExitStack() as ctx2:
                psS = ctx2.enter_context(
                    tc.tile_pool(name="psS", bufs=2, space="PSUM")
                )
                psV = ctx2.enter_context(
                    tc.tile_pool(name="psV", bufs=1, space="PSUM")
                )
                psO = ctx2.enter_context(
                    tc.tile_pool(name="psO", bufs=2, space="PSUM")
                )
                psO15 = ctx2.enter_context(
                    tc.tile_pool(name="psO15", bufs=1, space="PSUM")
                )

                # ones column of v_aug, once (region disjoint from v copies)
                nc.vector.memset(v_sb[:, :, 128:129], 1.0)

                out_ap = out_h[:]

                # Causal-packed wT layout: segment for k-block j holds
                # q in [j*128, T) at packed offset OFF[j].
                OFF = [0] * (TB + 1)
                for j in range(TB):
                    OFF[j + 1] = OFF[j] + (T - j * P)
                TOTAL = OFF[TB]  # 17408

                def wT_at(jj, qstart, width):
                    o = OFF[jj] + (qstart - jj * P)
                    return wT_sb[:, o : o + width]

                def emit_v(j):
                    pv = psV.tile([P, H], f32, tag="psV")
                    for c in range(DC):
                        nc.tensor.matmul(
                            pv,
                            xT_sb[:, c, j * P : (j + 1) * P],
                            wv_sb[:, c, :],
                            start=(c == 0),
                            stop=(c == DC - 1),
                        )
                    nc.vector.tensor_copy(v_sb[:, j, 0:H], pv)

                def emit_epilogue(i, po):
                    nc.vector.reciprocal(rec_all[:, i : i + 1], po[:, 128:129])
                    m = nc.scalar.mul(
                        ot_all[:, i, :], po[:, 0:H], rec_all[:, i : i + 1]
                    )
                    mul_names.append(m.ins.name)
                    for a, b in ((0, 4), (4, 8), (8, 12), (12, 16)):
                        if i == b - 1:
                            # rearrange so DMA iteration order (p, blk, h) on
                            # the SBUF side pairs with row blk*128+p in DRAM
                            nc.sync.dma_start(
                                out=out_ap[a * P : b * P, :].rearrange(
                                    "(b p) h -> p b h", p=P
                                ),
                                in_=ot_all[:, a:b, :],
                            )

                def emit_c_group(i):
                    po = psO.tile([P, 132], f32, tag="psO", name=f"po{i}")
                    for jj in range(i):
                        nc.tensor.matmul(
                            po[:, 0:129],
                            wT_at(jj, i * P, P),
                            v_sb[:, jj, 0:129],
                            start=(jj == 0),
                            stop=False,
                        )
                    nc.tensor.matmul(
                        po[:, 0:129],
                        dw_sb[:, i, :],
                        v_sb[:, i, 0:129],
                        start=(i == 0),
                        stop=True,
                    )
                    emit_epilogue(i, po)

                # exp pieces: 16x1024, then the last 1024 split at segment
                # boundaries (640 | 256 | 128) so late epilogues fire early.
                pieces = [(p * 1024, 1024) for p in range(16)]
                pieces += [(16384, 640), (17024, 256), (17280, 128)]

                po15 = psO15.tile([P, 132], f32, tag="po15")
                next15 = 0  # next C(15) k-block whose matmul is pending
                pending_c = None
                next_done = 0  # next j whose dw/epilogue trigger is pending

                # v_0/v_1 ahead of everything: zero-dep PE filler covering
                # the qT/kT eviction latency at the A1->B boundary.
                emit_v(0)
                emit_v(1)

                for pi, (ts, tw) in enumerate(pieces):
                    ps = psS.tile([P, 1024], f32, tag="psS")
                    # score matmuls covering packed [ts, ts+tw): split at
                    # PSUM bank boundaries, segment boundaries, and the
                    # qT lo|hi tile boundary (q = 1024).
                    for j in range(TB):
                        lo = max(ts, OFF[j])
                        hi = min(ts + tw, OFF[j + 1])
                        a = lo
                        while a < hi:
                            b = min(hi, ts + ((a - ts) // 512 + 1) * 512)
                            qg = j * P + (a - OFF[j])
                            if qg < 1024 < qg + (b - a):
                                b = a + (1024 - qg)
                            qt = qT_lo if qg < 1024 else qT_hi
                            nc.tensor.matmul(
                                ps[:, a - ts : b - ts],
                                kT_block(j),
                                qt[:, qg % 1024 : qg % 1024 + (b - a)],
                                start=True,
                                stop=True,
                            )
                            a = b
                    # C(15) accumulation: k-blocks whose segment is fully
                    # exp'd (emitted pre-exp so the whole-tile wT wait
                    # resolves to the previous piece's exp).
                    while next15 < TB - 1 and OFF[next15 + 1] <= ts:
                        nc.tensor.matmul(
                            po15[:, 0:129],
                            wT_at(next15, (TB - 1) * P, P),
                            v_sb[:, next15, 0:129],
                            start=(next15 == 0),
                            stop=False,
                        )
                        next15 += 1
                    # delayed C group: its wT dependency is the PREVIOUS exp
                    if pending_c is not None:
                        emit_c_group(pending_c)
                        pending_c = None
                    nc.scalar.activation(
                        wT_sb[:, ts : ts + tw], ps[:, :tw], EXP, scale=SCALE
                    )
                    if pi <= 13:
                        emit_v(pi + 2)
                    # epilogue trigger j: segment j's first 128 cols exp'd
                    while next_done < TB and OFF[next_done] + P <= ts + tw:
                        j = next_done
                        nc.vector.tensor_tensor(
                            dw_sb[:, j, :], wT_at(j, j * P, P), mask2_sb, MULT
                        )
                        if j > 0:
                            if pending_c is not None:
                                emit_c_group(pending_c)
                            pending_c = j - 1
                        next_done += 1

                # tail: C(14), then C(15)'s last off-diag + masked diagonal
                if pending_c is not None:
                    emit_c_group(pending_c)  # C(14)
                i15 = TB - 1
                while next15 < TB - 1:
                    nc.tensor.matmul(
                        po15[:, 0:129],
                        wT_at(next15, i15 * P, P),
                        v_sb[:, next15, 0:129],
                        start=(next15 == 0),
                        stop=False,
                    )
                    next15 += 1
                nc.tensor.matmul(
                    po15[:, 0:129],
                    dw_sb[:, i15, :],
                    v_sb[:, i15, 0:129],
                    start=False,
                    stop=True,
                )
                emit_epilogue(i15, po15)

    _strip_self_waits(nc)
    _strip_mul_pe_waits(nc, mul_names)
    nc.finalize()  # Bacc.compile(): wait legalization + register allocation
    return nc


def _strip_self_waits(nc):
    """Drop same-engine semaphore waits on in-order engines (PE/ACT/DVE
    execute and complete strictly in order, so a self-wait is redundant).
    Tile emits them conservatively; walrus allows only one sem wait per
    compute instruction, and these push some matmuls/tensor-ops over."""
    prefixes = {"PE": "PE_", "Activation": "Activation_", "DVE": "DVE_"}
    for bb in nc.m.functions[0].blocks:
        for inst in bb.instructions:
            si = inst.sync_info
            if not si or not si.on_wait:
                continue
            pref = prefixes.get(str(inst.engine).split(".")[-1])
            if pref is None:
                continue
            keep = [w for w in si.on_wait if not (w.ant_name or "").startswith(pref)]
            if len(keep) != len(si.on_wait):
                si.on_wait = keep
                inst.sync_info = si


def _strip_mul_pe_waits(nc, mul_names):
    """The out-scale mul waits {PE(po stop-matmul), DVE(reciprocal)}; the
    reciprocal itself waits on that same PE stop-matmul, so the mul's PE
    wait is transitively implied by its DVE wait. Dropping it leaves one
    wait -> no event-semaphore junction (each junction costs ~115ns per
    engine in the NEFF teardown sweep)."""
    names = set(mul_names)
    for bb in nc.m.functions[0].blocks:
        for inst in bb.instructions:
            if getattr(inst, "name", None) not in names:
                continue
            si = inst.sync_info
            if not si or not si.on_wait:
                continue
            keep = [w for w in si.on_wait if (w.ant_name or "").startswith("DVE")]
            if keep and len(keep) != len(si.on_wait):
                si.on_wait = keep
                inst.sync_info = si


def kernel(**inputs):
    global LAST_RESULT
    x = np.asarray(inputs["x"], dtype=np.float32)
    bf = ml_dtypes.bfloat16
    w_bf = {
        k: np.ascontiguousarray(
            np.asarray(inputs[k], dtype=np.float32)
            .astype(bf)
            .reshape(DC, P, H)
            .transpose(1, 0, 2)
        )
        for k in ("Wq", "Wk", "Wv")
    }
    # dw[p=k_local, f=q_local] keeps entries with k <= q
    mask01 = (
        (np.arange(P)[:, None] <= np.arange(P)[None, :]).astype(np.float32).astype(bf)
    )

    if "nc" not in _CACHE:
        _CACHE["nc"] = _build()
    nc = _CACHE["nc"]

    from concourse.bass_utils import run_bass_kernel_spmd

    in_maps = [
        {
            "xT": np.ascontiguousarray(x[b].T).astype(bf),
            "Wq": w_bf["Wq"],
            "Wk": w_bf["Wk"],
            "Wv": w_bf["Wv"],
            "mask": mask01,
        }
        for b in range(B)
    ]
    res = run_bass_kernel_spmd(nc, in_maps, core_ids=list(range(B)))
    LAST_RESULT = res
    return np.stack([res.results[b]["out"] for b in range(B)]).astype(np.float32)
